# revision 12
# baseline (speedup 1.0000x reference)
"""DeepseekV2 MLA decode (matrix-absorbed) on 8 Trainium2 NeuronCores.

bf16 everywhere (validated ~5e-3 rel err vs 2e-2 gate). Sharding:
  - W_DQ row-sharded (contraction) -> partial cQ -> AllReduce (49KB fp32) ->
    RMSNorm computed redundantly per core (ln_w folded into W_QR/W_UQ_UK).
  - W_QR / W_UQ_UK head-sharded (16 of 128 heads per core); q deinterleave
    folded into a host-side column permutation of W_QR.
  - AllGather of q (per-core [8,16,576] bf16).
  - Attention sharded over kv_len (1024 of 8192 positions per core, all 128
    heads); k roped on the HOST with relative positions (q stays un-roped);
    exp without max subtraction; partial (attn, lsum) ReduceScatter(add)
    grouped by head-block.
  - W_UV_O row-sharded (same 16 heads); final AllReduce of [8,5120] fp32.

Engine/queue split (each engine queue is in-order; a stalled DMA blocks
everything behind it on that queue):
  - SP (nc.sync): weight streams only (wdq, wqr, wuk, wuvo) - stall only on
    their own pool bufs, so the big wuvo stream is never blocked by
    collective-dependent loads.
  - Activation (nc.scalar): cache streams (ckv, keT) + exp/copies.
  - Pool (nc.gpsimd): collectives and everything ordered around them.
All weights/caches host-packed so every big DMA is contiguous per partition.
"""
import sys

if "/opt/trn_rl_repo" not in sys.path:
    sys.path.insert(0, "/opt/trn_rl_repo")

import numpy as np

N_CORES = 8
B = 8           # batch
H = 5120        # hidden
NH = 128        # heads
QLR = 1536      # q lora rank
ROPE = 64
KVLR = 512
KV = 8192
THETA = 10000.0
SCALE = 192.0 ** -0.5

HL = NH // N_CORES      # 16 local heads
KVL = KV // N_CORES     # 1024 local kv positions
HD = H // N_CORES       # 640 local hidden (stage-1 contraction shard)
KT = KVL // 128         # 8 kv tiles of 128 per core
QD = KVLR + ROPE        # 576 packed q dim

_CACHE = {}


def build_nc(sim=False):
    import concourse.bacc as bacc
    import concourse.mybir as mybir
    import concourse.tile as tile

    F32 = mybir.dt.float32
    BF16 = mybir.dt.bfloat16
    AF = mybir.ActivationFunctionType

    nc = bacc.Bacc("TRN2", target_bir_lowering=False, debug=False,
                   num_devices=(1 if sim else N_CORES))

    # ---- per-core inputs (host-packed layouts, see make_in_maps) ----
    hs = nc.dram_tensor("hs", [B, HD], BF16, kind="ExternalInput")
    wdq = nc.dram_tensor("wdq", [128, 5 * QLR], BF16, kind="ExternalInput")
    wqr = nc.dram_tensor("wqr", [128, 12 * HL * ROPE], BF16, kind="ExternalInput")
    wuk = nc.dram_tensor("wuk", [16 * 128, 12 * 512], BF16, kind="ExternalInput")
    ckv = nc.dram_tensor("ckv", [B, 128, KT * KVLR], BF16, kind="ExternalInput")
    ket = nc.dram_tensor("ket", [B, ROPE, KVL], BF16, kind="ExternalInput")
    ident = nc.dram_tensor("ident", [128, 128], BF16, kind="ExternalInput")
    wuvo = nc.dram_tensor("wuvo", [2 * 64 * 128, H // 2], BF16, kind="ExternalInput")
    out = nc.dram_tensor("out", [B, H], F32, kind="ExternalOutput")

    RG = [list(range(N_CORES))]

    def coll(kind, op, in_t, out_t):
        if not sim:
            nc.gpsimd.collective_compute(kind, op, replica_groups=RG,
                                         ins=[in_t.opt()], outs=[out_t.opt()])
        elif kind == "AllGather":
            nc.gpsimd.dma_start(out_t[0], in_t[:])
        elif kind == "ReduceScatter":
            nc.gpsimd.dma_start(out_t[:], in_t[0])
        else:
            nc.gpsimd.dma_start(out_t[:], in_t[:])

    NHALF = H // 2

    with tile.TileContext(nc) as tc:
        with (
            tc.tile_pool(name="const", bufs=1) as cpool,
            tc.tile_pool(name="dram", bufs=1, space="DRAM") as dram,
            tc.tile_pool(name="wuvo_sb", bufs=6) as wvp,
            tc.tile_pool(name="tpack", bufs=2, space="PSUM") as tpp,
            tc.tile_pool(name="cache", bufs=3) as cap,
            tc.tile_pool(name="s3sb", bufs=2) as s3,
        ):
            idt = cpool.tile([128, 128], BF16)
            nc.scalar.dma_start(idt[:], ident[:])
            eps = cpool.tile([8, 1], F32)
            nc.vector.memset(eps[:], 1e-6)
            hs_sb = cpool.tile([B, HD], BF16)
            nc.scalar.dma_start(hs_sb[:], hs[:])

            # collective bounce buffers
            cq_ar_in = dram.tile([B, QLR], F32)
            cq_ar_out = dram.tile([B, QLR], F32)
            q_ag_in = dram.tile([B, HL, QD], BF16)
            q_ag_out = dram.tile([N_CORES, B, HL, QD], BF16)
            at_rs_in = dram.tile([N_CORES, B, HL, KVLR + 1], F32)
            at_rs_out = dram.tile([B, HL, KVLR + 1], F32)
            o_ar_in = dram.tile([B, H], F32)
            o_ar_out = dram.tile([B, H], F32)

            # prefetch caches for b=0,1 before anything else on the Act queue
            ckv_t = [None] * B
            ket_t = [None] * B

            def load_cache(b):
                ckv_t[b] = cap.tile([128, KT, KVLR], BF16, tag="ckv", name=f"ckv{b}")
                nc.scalar.dma_start(ckv_t[b][:], ckv[b].rearrange("p (t l) -> p t l", t=KT))
                ket_t[b] = cap.tile([ROPE, KVL], BF16, tag="ket", name=f"ket{b}")
                nc.scalar.dma_start(ket_t[b][:], ket[b])

            load_cache(0)
            load_cache(1)

            # =========== Stage 1: cQ = rmsnorm(hs @ W_DQ) ===========
            with (
                tc.tile_pool(name="s12", bufs=1) as s1,
                tc.tile_pool(name="s12ps", bufs=1, space="PSUM") as s1ps,
                tc.tile_pool(name="qnps", bufs=2, space="PSUM") as qnps,
                tc.tile_pool(name="wuk_sb", bufs=2) as wkp,
            ):
                wdq_sb = s1.tile([128, 5, QLR], BF16)
                for k in range(5):
                    nc.sync.dma_start(
                        wdq_sb[:, k, :], wdq[:, k * QLR:(k + 1) * QLR])
                hsT = s1.tile([128, 5, 8], BF16)
                for k in range(5):
                    tp = tpp.tile([128, 8], BF16, tag="tp")
                    nc.tensor.transpose(tp[:], hs_sb[:, k * 128:(k + 1) * 128], idt[0:8, 0:8])
                    nc.vector.tensor_copy(hsT[:, k, :], tp[:])
                cq_ps = s1ps.tile([8, QLR], F32)
                for n in range(3):
                    for k in range(5):
                        nc.tensor.matmul(
                            cq_ps[:, n * 512:(n + 1) * 512],
                            hsT[:, k, :],
                            wdq_sb[:, k, n * 512:(n + 1) * 512],
                            start=(k == 0), stop=(k == 4),
                        )
                cqraw = s1.tile([8, QLR], F32)
                nc.vector.tensor_copy(cqraw[:], cq_ps[:])
                nc.gpsimd.dma_start(cq_ar_in[:], cqraw[:])
                coll("AllReduce", mybir.AluOpType.add, cq_ar_in, cq_ar_out)
                cqsum = s1.tile([8, QLR], F32)
                nc.gpsimd.dma_start(cqsum[:], cq_ar_out[:])
                # rmsnorm (ln_w folded into the weights host-side)
                sq = s1.tile([8, QLR], F32)
                ssq = s1.tile([8, 1], F32)
                nc.scalar.activation(sq[:], cqsum[:], AF.Square, accum_out=ssq[:])
                sdev = s1.tile([8, 1], F32)
                nc.scalar.activation(sdev[:], ssq[:], AF.Sqrt, bias=eps[:], scale=1.0 / QLR)
                rinv = s1.tile([8, 1], F32)
                nc.vector.reciprocal(rinv[:], sdev[:])
                cqn = s1.tile([8, QLR], BF16)
                nc.vector.tensor_scalar_mul(cqn[:], cqsum[:], rinv[:])
                cqnT = s1.tile([128, 12, 8], BF16)
                for k in range(12):
                    tp = tpp.tile([128, 8], BF16, tag="tp")
                    nc.tensor.transpose(tp[:], cqn[:, k * 128:(k + 1) * 128], idt[0:8, 0:8])
                    nc.vector.tensor_copy(cqnT[:, k, :], tp[:])

                # =========== Stage 2: q projections for 16 local heads ===========
                wqr_sb = s1.tile([128, 12, HL * ROPE], BF16)
                nc.sync.dma_start(
                    wqr_sb[:], wqr[:].rearrange("p (k n) -> p k n", k=12))
                qpe_sb = s1.tile([8, HL * ROPE], BF16)
                for n in range(2):
                    ps_q = qnps.tile([8, 512], F32, tag="psq")
                    for k in range(12):
                        nc.tensor.matmul(ps_q[:], cqnT[:, k, :],
                                         wqr_sb[:, k, n * 512:(n + 1) * 512],
                                         start=(k == 0), stop=(k == 11))
                    nc.vector.tensor_copy(qpe_sb[:, n * 512:(n + 1) * 512], ps_q[:])
                qn_sb = s1.tile([8, HL * KVLR], BF16)
                for nb in range(16):
                    wt = wkp.tile([128, 12, 512], BF16, tag="wuk")
                    nc.sync.dma_start(
                        wt[:], wuk[nb * 128:(nb + 1) * 128, :].rearrange(
                            "p (k n) -> p k n", k=12))
                    ps_q = qnps.tile([8, 512], F32, tag="psq")
                    for k in range(12):
                        nc.tensor.matmul(ps_q[:], cqnT[:, k, :],
                                         wt[:, k, :],
                                         start=(k == 0), stop=(k == 11))
                    nc.vector.tensor_copy(
                        qn_sb[:, nb * 512:(nb + 1) * 512], ps_q[:])
                # pack q into the allgather buffer
                nc.gpsimd.dma_start(
                    q_ag_in[:, :, 0:KVLR],
                    qn_sb[:].rearrange("b (h l) -> b h l", h=HL),
                )
                nc.gpsimd.dma_start(
                    q_ag_in[:, :, KVLR:QD],
                    qpe_sb[:].rearrange("b (h r) -> b h r", h=HL),
                )
                coll("AllGather", mybir.AluOpType.bypass, q_ag_in, q_ag_out)

            # =========== Stage 3: attention over local kv shard, all 128 heads ===========
            with (
                tc.tile_pool(name="s3ps", bufs=2, space="PSUM") as scps,
                tc.tile_pool(name="atps", bufs=2, space="PSUM") as atps,
            ):
                for b in range(B):
                    if b + 2 < B:
                        load_cache(b + 2)
                    ckv_sb = ckv_t[b]
                    # q for this batch (after AG)
                    qx = s3.tile([128, QD], BF16, tag="qx")
                    nc.gpsimd.dma_start(qx[:], q_ag_out[:, b, :, :])
                    # transpose q for this batch
                    qnT = s3.tile([128, 4, 128], BF16, tag="qnT")
                    tp = tpp.tile([128, 512], BF16, tag="tp")
                    for lc in range(4):
                        nc.tensor.transpose(tp[:, lc * 128:(lc + 1) * 128],
                                            qx[:, lc * 128:(lc + 1) * 128], idt[:])
                    nc.vector.tensor_copy(qnT[:].rearrange("p a c -> p (a c)"), tp[:])
                    qeT = s3.tile([64, 128], BF16, tag="qeT")
                    tpq = tpp.tile([64, 128], BF16, tag="tp")
                    nc.tensor.transpose(tpq[:], qx[:, KVLR:QD], idt[:])
                    nc.vector.tensor_copy(qeT[:], tpq[:])
                    # transpose ckv -> ckvT [l=4x128, kv=KVL]
                    ckvT = s3.tile([128, 4, KVL], BF16, tag="ckvT")
                    for lc in range(4):
                        for g in range(KT // 4):
                            tp = tpp.tile([128, 512], BF16, tag="tp")
                            for j in range(4):
                                t = g * 4 + j
                                nc.tensor.transpose(tp[:, j * 128:(j + 1) * 128],
                                                    ckv_sb[:, t, lc * 128:(lc + 1) * 128],
                                                    idt[:])
                            nc.vector.tensor_copy(ckvT[:, lc, g * 512:(g + 1) * 512], tp[:])
                    # scores = qn . ckv^T + qe . ke^T   [128h, KVL]
                    sc_ps = scps.tile([128, KVL], F32, tag="sc")
                    for c in range(KVL // 512):
                        sl = slice(c * 512, (c + 1) * 512)
                        for lc in range(4):
                            nc.tensor.matmul(sc_ps[:, sl], qnT[:, lc, :], ckvT[:, lc, sl],
                                             start=(lc == 0), stop=False)
                        nc.tensor.matmul(sc_ps[:, sl], qeT[:], ket_t[b][:, sl],
                                         start=False, stop=True)
                    # probs (unnormalized) + partial lsum
                    attn_sb = s3.tile([128, KVLR + 1], F32, tag="attn")
                    probs = s3.tile([128, KVL], BF16, tag="probs")
                    nc.scalar.activation(probs[:], sc_ps[:], AF.Exp, scale=SCALE,
                                         accum_out=attn_sb[:, KVLR:KVLR + 1])
                    # probs^T
                    probsT = s3.tile([128, KT, 128], BF16, tag="probsT")
                    for g in range(KT // 4):
                        tp = tpp.tile([128, 512], BF16, tag="tp")
                        for j in range(4):
                            t = g * 4 + j
                            nc.tensor.transpose(tp[:, j * 128:(j + 1) * 128],
                                                probs[:, t * 128:(t + 1) * 128], idt[:])
                        nc.vector.tensor_copy(
                            probsT[:, g * 4:(g + 1) * 4, :].rearrange("p a c -> p (a c)"),
                            tp[:])
                    # attn partial = probs^T . ckv  [128h, KVLR]
                    at_ps = atps.tile([128, KVLR], F32, tag="at")
                    for t in range(KT):
                        nc.tensor.matmul(at_ps[:], probsT[:, t, :], ckv_sb[:, t, :],
                                         start=(t == 0), stop=(t == KT - 1))
                    nc.vector.tensor_copy(attn_sb[:, 0:KVLR], at_ps[:])
                    # scatter into reduce buffer grouped by head-block
                    nc.gpsimd.dma_start(at_rs_in[:, b, :, :], attn_sb[:])
                coll("ReduceScatter", mybir.AluOpType.add, at_rs_in, at_rs_out)

            # =========== Stage 4: out = (attn/lsum) @ W_UV_O, head shard ===========
            with (
                tc.tile_pool(name="s4", bufs=1) as s4,
                tc.tile_pool(name="oaps", bufs=1, space="PSUM") as oaps,
            ):
                o_sb = s4.tile([8, HL, KVLR + 1], F32)
                nc.gpsimd.dma_start(o_sb[:], at_rs_out[:])
                linv = s4.tile([8, HL], F32)
                nc.vector.reciprocal(linv[:], o_sb[:, :, KVLR])
                osc = s4.tile([8, HL, KVLR], BF16)
                for h in range(HL):
                    nc.vector.tensor_scalar_mul(osc[:, h, :], o_sb[:, h, 0:KVLR],
                                                linv[:, h:h + 1])
                aT = s4.tile([128, HL * 4, 8], BF16)
                for h in range(HL):
                    tp = tpp.tile([128, 32], BF16, tag="tp")
                    for lc in range(4):
                        nc.tensor.transpose(tp[:, lc * 8:(lc + 1) * 8],
                                            osc[:, h, lc * 128:(lc + 1) * 128],
                                            idt[0:8, 0:8])
                    nc.vector.tensor_copy(
                        aT[:, h * 4:(h + 1) * 4, :].rearrange("p a c -> p (a c)"), tp[:])
                outp = s4.tile([8, H], F32)
                for half in range(2):
                    o_ps = oaps.tile([8, NHALF], F32, tag="ops")
                    for r in range(64):
                        wt = wvp.tile([128, NHALF], BF16, tag="wuvo")
                        nc.sync.dma_start(
                            wt[:], wuvo[(half * 64 + r) * 128:(half * 64 + r + 1) * 128, :])
                        for n5 in range(NHALF // 512):
                            nc.tensor.matmul(o_ps[:, n5 * 512:(n5 + 1) * 512],
                                             aT[:, r, :], wt[:, n5 * 512:(n5 + 1) * 512],
                                             start=(r == 0), stop=(r == 63))
                    nc.scalar.copy(outp[:, half * NHALF:(half + 1) * NHALF], o_ps[:])
                nc.gpsimd.dma_start(o_ar_in[:], outp[:])
                coll("AllReduce", mybir.AluOpType.add, o_ar_in, o_ar_out)
                nc.gpsimd.dma_start(out[:], o_ar_out[:])

    nc.compile()
    return nc


def make_in_maps(hidden_states, compressed_kv_normed_cache, k_pe_cache,
                 W_DQ, ln_w, W_QR, W_UQ_UK, W_UV_O):
    import ml_dtypes
    f32 = np.float32
    bf16 = ml_dtypes.bfloat16

    hidden_states = np.asarray(hidden_states, f32)
    ckv = np.asarray(compressed_kv_normed_cache, f32)
    kpe = np.asarray(k_pe_cache, f32)
    W_DQ = np.asarray(W_DQ, f32)
    ln_w = np.asarray(ln_w, f32)
    W_QR = np.asarray(W_QR, f32) * ln_w[:, None]
    W_UQ_UK = np.asarray(W_UQ_UK, f32) * ln_w[:, None]
    W_UV_O = np.asarray(W_UV_O, f32)

    # fold the rope-pair deinterleave of q into W_QR's columns:
    # new col (h, j) = old col (h, 2j) for j<32, (h, 2(j-32)+1) for j>=32
    perm = np.arange(NH * ROPE).reshape(NH, ROPE)
    perm = np.concatenate([perm[:, 0::2], perm[:, 1::2]], axis=1).reshape(-1)
    W_QR = W_QR[:, perm]

    # host-side rope of the k cache with *relative* positions (q un-roped),
    # deinterleaved into halves; then transposed to [rope, kv]
    inv = 1.0 / (THETA ** (np.arange(0, ROPE, 2, dtype=np.float64) / ROPE))
    rel = (np.arange(KV, dtype=np.float64) - (KV - 1))[:, None] * inv[None, :]
    cost = np.cos(rel).astype(f32)
    sint = np.sin(rel).astype(f32)
    k0, k1 = kpe[..., 0::2], kpe[..., 1::2]
    ke = np.concatenate([k0 * cost - k1 * sint, k0 * sint + k1 * cost],
                        axis=-1)                      # [B, KV, ROPE]
    keT = ke.transpose(0, 2, 1)                       # [B, ROPE, KV]

    ident = np.eye(128, dtype=bf16)

    def c(x):
        return np.ascontiguousarray(x.astype(bf16))

    in_maps = []
    for ci in range(N_CORES):
        # wdq: [HD, QLR] -> [128 p, 5 k, QLR]
        wdq_p = W_DQ[ci * HD:(ci + 1) * HD, :].reshape(5, 128, QLR) \
            .transpose(1, 0, 2).reshape(128, 5 * QLR)
        # wqr: [QLR, HL*ROPE] -> [128 p, 12 k, 1024 n]
        wqr_p = W_QR[:, ci * HL * ROPE:(ci + 1) * HL * ROPE] \
            .reshape(12, 128, HL * ROPE).transpose(1, 0, 2).reshape(128, -1)
        # wuk: [QLR, HL*KVLR] -> [16 nb, 128 p, 12 k, 512 nw] -> [2048, 6144]
        wuk_p = W_UQ_UK[:, ci * HL * KVLR:(ci + 1) * HL * KVLR] \
            .reshape(12, 128, 16, 512).transpose(2, 1, 0, 3).reshape(16 * 128, -1)
        # ckv: [B, KVL, KVLR] -> [B, 128 p, KT t, KVLR]
        ckv_p = ckv[:, ci * KVL:(ci + 1) * KVL, :] \
            .reshape(B, KT, 128, KVLR).transpose(0, 2, 1, 3).reshape(B, 128, -1)
        # keT: [B, ROPE, KVL] slice
        ket_p = keT[:, :, ci * KVL:(ci + 1) * KVL]
        # wuvo: [HL*KVLR, H] -> [2 half, 64 r, 128 p, 2560] -> [16384, 2560]
        wuvo_p = W_UV_O[ci * HL * KVLR:(ci + 1) * HL * KVLR, :] \
            .reshape(64, 128, 2, H // 2).transpose(2, 0, 1, 3).reshape(-1, H // 2)
        in_maps.append({
            "hs": c(hidden_states[:, ci * HD:(ci + 1) * HD]),
            "wdq": c(wdq_p),
            "wqr": c(wqr_p),
            "wuk": c(wuk_p),
            "ckv": c(ckv_p),
            "ket": c(ket_p),
            "ident": ident,
            "wuvo": c(wuvo_p),
        })
    return in_maps


def kernel(**inputs) -> np.ndarray:
    from concourse import bass_utils

    if "nc" not in _CACHE:
        _CACHE["nc"] = build_nc()
    nc = _CACHE["nc"]
    in_maps = make_in_maps(**inputs)
    res = bass_utils.run_bass_kernel_spmd(nc, in_maps, core_ids=list(range(N_CORES)))
    return np.asarray(res.results[0]["out"], np.float32)
